# revision 6
# baseline (speedup 1.0000x reference)
"""Causal self-attention kernel for 8 Trainium2 NeuronCores.

Problem: B=4, T=2048, C=1024, NH=16, HD=64 (fp32).

Sharding: core c = (batch b = c//2, head-group g = c%2 of 8 heads).
Per core, everything is computed in transposed layout so no on-device
transposes are needed:
  - host supplies xT = x[b].T [C, T], plus head-group-sliced/permuted
    weights (column-parallel W_attn, row-parallel W_proj)
  - qT/kT [feat, tok] via W stationary / xT moving; v [tok, feat] via
    xT stationary / W_v moving, with a fused ones-column per head so the
    attention row-sum (softmax denominator) falls out of the same matmul
  - scores are computed transposed [keys, queries] per 128-key block;
    exp on ACT writes bf16; causal masking = one [128,128] triangle
    multiply on the diagonal sub-block only; columns left of the diagonal
    sub-block (entirely masked) are clipped out of the score/exp/AV APs;
    blocks entirely above the diagonal are skipped
  - y^T accumulates in PSUM over key blocks; normalized by 1/Z where the
    1/Z row is partition-broadcast on GPSIMD (no DRAM round trip)
  - output projection is row-parallel -> partial out^T [C, T] DMA'd
    straight to DRAM; the pairwise reduction (+transpose) happens on host.

Matmuls run fp32r (fp22, full PE rate); q/k/esb/v are bf16.
"""

import numpy as np
import ml_dtypes
from contextlib import ExitStack

import concourse.bass as bass
import concourse.tile as tile
import concourse.mybir as mybir
from concourse import bacc
from concourse.bass_utils import run_bass_kernel_spmd

B, C, NH, HD = 4, 1024, 16, 64
NCORES = 8
NP = 4              # head pairs per core (8 heads)
QC = 512            # query-chunk (free dim of most matmuls)
KB = 128            # key block (partition dim of score blocks)
CCH = C // 128      # 8 contraction chunks
FP32 = mybir.dt.float32
FP32R = mybir.dt.float32r
BF16 = mybir.dt.bfloat16
EXP = mybir.ActivationFunctionType.Exp


def build_program(T=2048):
    nqc = T // QC
    nc = bacc.Bacc("TRN2", target_bir_lowering=False, debug=False,
                   num_devices=NCORES)

    xt_d = nc.dram_tensor("xt", [C, T], FP32, kind="ExternalInput").ap()
    wqk_d = nc.dram_tensor("wqk", [C, C], FP32, kind="ExternalInput").ap()
    bqk_d = nc.dram_tensor("bqk", [C], FP32, kind="ExternalInput").ap()
    wv_d = nc.dram_tensor("wv", [C, 512], FP32, kind="ExternalInput").ap()
    bv_d = nc.dram_tensor("bv", [512], FP32, kind="ExternalInput").ap()
    wp_d = nc.dram_tensor("wp", [512, C], FP32, kind="ExternalInput").ap()
    bp_d = nc.dram_tensor("bp", [C], FP32, kind="ExternalInput").ap()
    mask_d = nc.dram_tensor("mask", [128, KB], BF16, kind="ExternalInput").ap()
    out_d = nc.dram_tensor("out_t", [C, T], FP32, kind="ExternalOutput").ap()
    rz_dr = [nc.dram_tensor(f"rzd{i}", [QC], FP32).ap() for i in range(4)]

    with tile.TileContext(nc) as tc, ExitStack() as ctx:
        resid = ctx.enter_context(tc.tile_pool(name="resid", bufs=1))
        xtp = ctx.enter_context(tc.tile_pool(name="xtp", bufs=2))
        qp = ctx.enter_context(tc.tile_pool(name="qp", bufs=2))
        yp = ctx.enter_context(tc.tile_pool(name="yp", bufs=2))
        ep = ctx.enter_context(tc.tile_pool(name="ep", bufs=3))
        sm = ctx.enter_context(tc.tile_pool(name="sm", bufs=3))
        op = ctx.enter_context(tc.tile_pool(name="op", bufs=2))
        ps_acc = ctx.enter_context(tc.tile_pool(name="ps_acc", bufs=1, space="PSUM"))
        ps_s = ctx.enter_context(tc.tile_pool(name="ps_s", bufs=2, space="PSUM"))
        ps_y = ctx.enter_context(tc.tile_pool(name="ps_y", bufs=3, space="PSUM"))

        # ---- residents: weights, biases, masks, k/v tiles
        wqk_sb = resid.tile([128, CCH, 8, 128], FP32R, name="wqk_sb")
        for cc in range(CCH):
            nc.sync.dma_start(out=wqk_sb[:, cc], in_=wqk_d[cc * 128:(cc + 1) * 128, :].rearrange("p (f n) -> p f n", f=8).bitcast(FP32R))
        wv_sb = resid.tile([128, CCH, 512], FP32R, name="wv_sb")
        for cc in range(CCH):
            nc.sync.dma_start(out=wv_sb[:, cc], in_=wv_d[cc * 128:(cc + 1) * 128, :].bitcast(FP32R))
        wp_sb = resid.tile([128, NP, 8, 128], FP32R, name="wp_sb")
        for p in range(NP):
            nc.sync.dma_start(out=wp_sb[:, p], in_=wp_d[p * 128:(p + 1) * 128, :].rearrange("p (f n) -> p f n", f=8).bitcast(FP32R))

        bqk_sb = resid.tile([128, 8], FP32, name="bqk_sb")
        nc.sync.dma_start(out=bqk_sb, in_=bqk_d.rearrange("(f p) -> p f", p=128))
        bp_sb = resid.tile([128, 8], FP32, name="bp_sb")
        nc.sync.dma_start(out=bp_sb, in_=bp_d.rearrange("(f p) -> p f", p=128))
        # bv_bc[:, l, :] = b_v (broadcast over partitions)
        bv_bc = resid.tile([128, 8, HD], FP32, name="bv_bc")
        nc.sync.dma_start(
            out=bv_bc,
            in_=bv_d.rearrange("(l d) -> l d", l=8).partition_broadcast(128))

        # causal triangle for the diagonal 128x128 sub-block:
        # mask[p, m] = 1.0 iff key-offset p <= query-offset m
        mask = resid.tile([128, KB], BF16, name="mask")
        nc.sync.dma_start(out=mask, in_=mask_d)

        ksb = [resid.tile([128, T], BF16, name=f"ksb{p}") for p in range(NP)]
        vsb = [resid.tile([128, 8, HD + 1], BF16, name=f"vsb{tb}")
               for tb in range(T // 128)]
        for tb in range(T // 128):
            # ones column rides along into v so the attention row-sum Z
            # falls out of the same AV matmul
            nc.vector.memset(vsb[tb][:, :, HD:HD + 1], 1.0)

        # ---------- emission helpers ----------
        def load_xt(qc, split=False):
            xt_sb = xtp.tile([128, CCH, QC], FP32R, name="xt_sb")
            src = xt_d[:, qc * QC:(qc + 1) * QC].rearrange(
                "(c p) f -> p c f", p=128).bitcast(FP32R)
            if split:
                for cc in range(CCH):
                    nc.scalar.dma_start(out=xt_sb[:, cc], in_=src[:, cc])
            else:
                nc.scalar.dma_start(out=xt_sb, in_=src)
            return xt_sb

        def emit_v(xt_sb, qc, j):
            tb = qc * (QC // 128) + j
            pv = ps_acc.tile([128, 512], FP32, name="pv")
            for cc in range(CCH):
                nc.tensor.matmul(
                    out=pv, lhsT=xt_sb[:, cc, j * 128:(j + 1) * 128],
                    rhs=wv_sb[:, cc], start=(cc == 0), stop=(cc == CCH - 1))
            nc.vector.tensor_add(
                vsb[tb][:, :, 0:HD],
                pv.rearrange("p (l d) -> p l d", l=8), bv_bc)

        def emit_qk(xt_sb, q_sb, qc, f):
            pqk = ps_acc.tile([128, QC], FP32, name="pqk", tag="pv")
            for cc in range(CCH):
                nc.tensor.matmul(
                    out=pqk, lhsT=wqk_sb[:, cc, f], rhs=xt_sb[:, cc],
                    start=(cc == 0), stop=(cc == CCH - 1))
            p, isk = f // 2, f % 2
            dst = (ksb[p][:, qc * QC:(qc + 1) * QC] if isk else q_sb[p])
            nc.vector.tensor_scalar_add(dst, pqk, bqk_sb[:, f:f + 1])

        def emit_proj(y_qc, qc, oc):
            pp = ps_acc.tile([128, QC], FP32, name="pp", tag="pv")
            for p in range(NP):
                nc.tensor.matmul(out=pp, lhsT=wp_sb[:, p, oc], rhs=y_qc[p],
                                 start=(p == 0), stop=(p == NP - 1))
            po = op.tile([128, QC], FP32, name="po")
            nc.vector.tensor_scalar_add(po, pp, bp_sb[:, oc:oc + 1])
            nc.sync.dma_start(
                out=out_d[oc * 128:(oc + 1) * 128, qc * QC:(qc + 1) * QC],
                in_=po)

        def new_q():
            return [qp.tile([128, QC], BF16, name=f"qsb{p}", tag=f"qsb{p}")
                    for p in range(NP)]

        # ---------- prologue: chunk 0 qkv projection ----------
        xt_cur = load_xt(0, split=True)
        q_cur = new_q()
        for f in range(8):
            emit_qk(xt_cur, q_cur, 0, f)
        for j in range(QC // 128):
            emit_v(xt_cur, 0, j)

        y_prev = None
        for qc in range(nqc):
            # background PE units interleaved into this chunk's attention:
            # next chunk's qkv projection + previous chunk's output projection
            bg = []
            if qc + 1 < nqc:
                xt_nxt = load_xt(qc + 1)
                q_nxt = new_q()
                for j in range(QC // 128):
                    bg.append((emit_v, (xt_nxt, qc + 1, j)))
                for f in range(8):
                    bg.append((emit_qk, (xt_nxt, q_nxt, qc + 1, f)))
            else:
                xt_nxt, q_nxt = None, None
            if y_prev is not None:
                for oc in range(8):
                    bg.append((emit_proj, (y_prev, qc - 1, oc)))
            bg_total = len(bg)

            njb = 4 * (qc + 1)
            steps = NP * njb
            y_cur = [yp.tile([128, QC], FP32R, name=f"y{p}", tag=f"y{p}")
                     for p in range(NP)]
            step = 0
            for p in range(NP):
                yps = [ps_y.tile([HD + 1, QC], FP32, name=f"yps{e}", tag="yps")
                       for e in (0, 1)]
                for jb in range(njb):
                    r = jb - 4 * qc
                    cs = KB * r if r > 0 else 0
                    sps = ps_s.tile([128, 2, QC], FP32, name="sps")
                    for e in (0, 1):
                        nc.tensor.matmul(
                            out=sps[:, e, cs:],
                            lhsT=ksb[p][e * HD:(e + 1) * HD,
                                        jb * KB:(jb + 1) * KB],
                            rhs=q_cur[p][e * HD:(e + 1) * HD, cs:],
                            start=True, stop=True)
                    esb = ep.tile([128, 2, QC], BF16, name="esb")
                    nc.scalar.activation(out=esb[:, :, cs:], in_=sps[:, :, cs:],
                                         func=EXP, scale=0.125)
                    if r >= 0:
                        for e in (0, 1):
                            nc.vector.tensor_mul(
                                esb[:, e, cs:cs + KB],
                                esb[:, e, cs:cs + KB], mask)
                    for e in (0, 1):
                        nc.tensor.matmul(
                            out=yps[e][:, cs:], lhsT=vsb[jb][:, 2 * p + e, :],
                            rhs=esb[:, e, cs:],
                            start=(jb == 0), stop=(jb == njb - 1))
                    # keep the in-order PE stream dense: spread background
                    # units evenly across the attention steps
                    step += 1
                    while bg and len(bg) > bg_total * (steps - step) // steps:
                        fn, args = bg.pop(0)
                        fn(*args)
                # normalize: rz = 1/Z, broadcast over HD partitions via a
                # DRAM round trip (the sync DMA queue is quiet now that the
                # collective is gone, so the latency hides under attention)
                for e in (0, 1):
                    rz = sm.tile([1, QC], FP32, name="rz")
                    nc.vector.tensor_copy(rz, yps[e][HD:HD + 1, :])
                    nc.vector.reciprocal_approx_fast(rz, rz)
                    rzb = sm.tile([HD, QC], FP32, name="rzb")
                    slot = rz_dr[2 * (p % 2) + e]
                    # scalar-engine HWDGE queue: stays clear of the weight
                    # loads and proj-output writes on the sync queue
                    nc.scalar.dma_start(out=slot, in_=rz[0:1, :])
                    nc.scalar.dma_start(out=rzb, in_=slot.partition_broadcast(HD))
                    nc.vector.tensor_mul(
                        y_cur[p][e * HD:(e + 1) * HD, :], yps[e][0:HD, :], rzb)
            for fn, args in bg:
                fn(*args)
            y_prev = y_cur
            xt_cur, q_cur = xt_nxt, q_nxt

        # epilogue: last chunk's projection
        for oc in range(8):
            emit_proj(y_prev, nqc - 1, oc)

    nc.compile()
    return nc


def shard_inputs(x, W_attn, b_attn, W_proj, b_proj):
    in_maps = []
    u = np.arange(KB)[None, :]
    p_ = np.arange(128)[:, None]
    mask_np = (p_ <= u).astype(ml_dtypes.bfloat16)
    for c in range(NCORES):
        b, g = c // 2, c % 2
        xt = np.ascontiguousarray(x[b].T.astype(np.float32))
        # w_qk columns: feat chunk f = 2p+isK holds q (isK=0) or k (isK=1)
        # features of heads (8g+2p, 8g+2p+1)
        qk_idx = []
        for f in range(8):
            p, isk = f // 2, f % 2
            for e in (0, 1):
                h = 8 * g + 2 * p + e
                base = isk * C + h * HD
                qk_idx.append(np.arange(base, base + HD))
        qk_idx = np.concatenate(qk_idx)
        v_idx = np.concatenate(
            [np.arange(2 * C + (8 * g + l) * HD, 2 * C + (8 * g + l) * HD + HD)
             for l in range(8)])
        p_idx = np.concatenate(
            [np.arange((8 * g + l) * HD, (8 * g + l) * HD + HD)
             for l in range(8)])
        in_maps.append({
            "mask": mask_np,
            "xt": xt,
            "wqk": np.ascontiguousarray(W_attn[:, qk_idx].astype(np.float32)),
            "bqk": np.ascontiguousarray(b_attn[qk_idx].astype(np.float32)),
            "wv": np.ascontiguousarray(W_attn[:, v_idx].astype(np.float32)),
            "bv": np.ascontiguousarray(b_attn[v_idx].astype(np.float32)),
            "wp": np.ascontiguousarray(W_proj[p_idx, :].astype(np.float32)),
            "bp": (b_proj.astype(np.float32) if g == 0
                   else np.zeros(C, np.float32)),
        })
    return in_maps


def assemble_output(results, T):
    out = np.empty((B, T, C), np.float32)
    for b in range(B):
        # each core holds a full [C, T] partial of out[b].T (row-parallel
        # projection); reduce the pair on host
        acc = results[2 * b]["out_t"] + results[2 * b + 1]["out_t"]
        out[b] = acc.T
    return out


_PROG = {}


def _get_program(T):
    if T not in _PROG:
        _PROG[T] = build_program(T)
    return _PROG[T]


def run_sharded(inputs, trace=False):
    """Returns (output [B,T,C], BassKernelResults)."""
    x = np.asarray(inputs["x"])
    T = x.shape[1]
    nc = _get_program(T)
    in_maps = shard_inputs(x, np.asarray(inputs["W_attn"]),
                           np.asarray(inputs["b_attn"]),
                           np.asarray(inputs["W_proj"]),
                           np.asarray(inputs["b_proj"]))
    res = run_bass_kernel_spmd(nc, in_maps, list(range(NCORES)), trace=trace)
    return assemble_output(res.results, T), res


def kernel(**inputs):
    out, _ = run_sharded(inputs)
    return out


# revision 11
# speedup vs baseline: 1.0053x; 1.0053x over previous
"""Causal self-attention kernel for 8 Trainium2 NeuronCores.

Problem: B=4, T=2048, C=1024, NH=16, HD=64 (fp32).

Sharding: core c = (batch b = c//2, head-group g = c%2 of 8 heads).
Per core, everything is computed in transposed layout so no on-device
transposes are needed:
  - host supplies xT = x[b].T [C, T], plus head-group-sliced/permuted
    weights (column-parallel W_attn, row-parallel W_proj)
  - qT/kT [feat, tok] via W stationary / xT moving; v [tok, feat] via
    xT stationary / W_v moving, with a fused ones-column per head so the
    attention row-sum (softmax denominator) falls out of the same matmul
  - scores are computed transposed [keys, queries] per 128-key block;
    exp on ACT writes bf16; causal masking = one [128,128] triangle
    multiply on the diagonal sub-block only; columns left of the diagonal
    sub-block (entirely masked) are clipped out of the score/exp/AV APs;
    blocks entirely above the diagonal are skipped
  - y^T accumulates in PSUM over key blocks; normalized by 1/Z where the
    1/Z row is partition-broadcast on GPSIMD (no DRAM round trip)
  - output projection is row-parallel -> partial out^T [C, T] DMA'd
    straight to DRAM; the pairwise reduction (+transpose) happens on host.

Matmuls run fp32r (fp22, full PE rate); q/k/esb/v are bf16.
"""

import numpy as np
import ml_dtypes
from contextlib import ExitStack

import concourse.bass as bass
import concourse.tile as tile
import concourse.mybir as mybir
from concourse import bacc
from concourse.bass_utils import run_bass_kernel_spmd

B, C, NH, HD = 4, 1024, 16, 64
NCORES = 8
NP = 4              # head pairs per core (8 heads)
QC = 512            # query-chunk (free dim of most matmuls)
KB = 128            # key block (partition dim of score blocks)
CCH = C // 128      # 8 contraction chunks
FP32 = mybir.dt.float32
FP32R = mybir.dt.float32r
BF16 = mybir.dt.bfloat16
EXP = mybir.ActivationFunctionType.Exp


def build_program(T=2048):
    nqc = T // QC
    nc = bacc.Bacc("TRN2", target_bir_lowering=False, debug=False,
                   num_devices=NCORES)

    xt_d = nc.dram_tensor("xt", [C, T], FP32, kind="ExternalInput").ap()
    # weights come host-pre-laid-out as [128, ...] so each loads as one
    # contiguous wide-line DMA (512B lines of the naive layout are ~4x
    # slower per the DMA efficiency curve)
    wqk_d = nc.dram_tensor("wqk", [128, CCH * 8 * 128], FP32, kind="ExternalInput").ap()
    bqk_d = nc.dram_tensor("bqk", [C], FP32, kind="ExternalInput").ap()
    wv_d = nc.dram_tensor("wv", [128, CCH * 512], FP32, kind="ExternalInput").ap()
    bv_d = nc.dram_tensor("bv", [512], FP32, kind="ExternalInput").ap()
    wp_d = nc.dram_tensor("wp", [128, NP * 8 * 128], FP32, kind="ExternalInput").ap()
    bp_d = nc.dram_tensor("bp", [C], FP32, kind="ExternalInput").ap()
    mask_d = nc.dram_tensor("mask", [128, KB], BF16, kind="ExternalInput").ap()
    out_d = nc.dram_tensor("out_t", [C, T], FP32, kind="ExternalOutput").ap()
    rz_dr = [nc.dram_tensor(f"rzd{i}", [QC], FP32).ap() for i in range(4)]

    with tile.TileContext(nc) as tc, ExitStack() as ctx:
        resid = ctx.enter_context(tc.tile_pool(name="resid", bufs=1))
        xtp = ctx.enter_context(tc.tile_pool(name="xtp", bufs=2))
        qp = ctx.enter_context(tc.tile_pool(name="qp", bufs=2))
        yp = ctx.enter_context(tc.tile_pool(name="yp", bufs=2))
        ep = ctx.enter_context(tc.tile_pool(name="ep", bufs=3))
        sm = ctx.enter_context(tc.tile_pool(name="sm", bufs=3))
        op = ctx.enter_context(tc.tile_pool(name="op", bufs=2))
        ps_acc = ctx.enter_context(tc.tile_pool(name="ps_acc", bufs=1, space="PSUM"))
        ps_s = ctx.enter_context(tc.tile_pool(name="ps_s", bufs=2, space="PSUM"))
        ps_y = ctx.enter_context(tc.tile_pool(name="ps_y", bufs=3, space="PSUM"))

        # ---- residents: weights, biases, masks, k/v tiles
        wqk_sb = resid.tile([128, CCH, 8, 128], FP32R, name="wqk_sb")
        nc.sync.dma_start(
            out=wqk_sb,
            in_=wqk_d.rearrange("p (c f n) -> p c f n", c=CCH, f=8).bitcast(FP32R))
        wv_sb = resid.tile([128, CCH, 512], FP32R, name="wv_sb")
        nc.sync.dma_start(
            out=wv_sb,
            in_=wv_d.rearrange("p (c n) -> p c n", c=CCH).bitcast(FP32R))
        wp_sb = resid.tile([128, NP, 8, 128], FP32R, name="wp_sb")
        nc.sync.dma_start(
            out=wp_sb,
            in_=wp_d.rearrange("p (g f n) -> p g f n", g=NP, f=8).bitcast(FP32R))

        bqk_sb = resid.tile([128, 8], FP32, name="bqk_sb")
        nc.sync.dma_start(out=bqk_sb, in_=bqk_d.rearrange("(f p) -> p f", p=128))
        bp_sb = resid.tile([128, 8], FP32, name="bp_sb")
        nc.sync.dma_start(out=bp_sb, in_=bp_d.rearrange("(f p) -> p f", p=128))
        # bv_bc[:, l, :] = b_v (broadcast over partitions)
        bv_bc = resid.tile([128, 8, HD], FP32, name="bv_bc")
        nc.sync.dma_start(
            out=bv_bc,
            in_=bv_d.rearrange("(l d) -> l d", l=8).partition_broadcast(128))

        # causal triangle for the diagonal 128x128 sub-block:
        # mask[p, m] = 1.0 iff key-offset p <= query-offset m
        mask = resid.tile([128, KB], BF16, name="mask")
        nc.sync.dma_start(out=mask, in_=mask_d)

        ksb = [resid.tile([128, T], BF16, name=f"ksb{p}") for p in range(NP)]
        vsb = [resid.tile([128, 8, HD + 1], BF16, name=f"vsb{tb}")
               for tb in range(T // 128)]
        for tb in range(T // 128):
            # ones column rides along into v so the attention row-sum Z
            # falls out of the same AV matmul
            nc.vector.memset(vsb[tb][:, :, HD:HD + 1], 1.0)

        # ---------- emission helpers ----------
        def load_xt(qc, split=False):
            xt_sb = xtp.tile([128, CCH, QC], FP32R, name="xt_sb")
            src = xt_d[:, qc * QC:(qc + 1) * QC].rearrange(
                "(c p) f -> p c f", p=128).bitcast(FP32R)
            if split:
                for cc in range(CCH):
                    nc.scalar.dma_start(out=xt_sb[:, cc], in_=src[:, cc])
            else:
                nc.scalar.dma_start(out=xt_sb, in_=src)
            return xt_sb

        def emit_v(xt_sb, qc, j):
            tb = qc * (QC // 128) + j
            pv = ps_acc.tile([128, 512], FP32, name="pv")
            for cc in range(CCH):
                nc.tensor.matmul(
                    out=pv, lhsT=xt_sb[:, cc, j * 128:(j + 1) * 128],
                    rhs=wv_sb[:, cc], start=(cc == 0), stop=(cc == CCH - 1))
            nc.vector.tensor_add(
                vsb[tb][:, :, 0:HD],
                pv.rearrange("p (l d) -> p l d", l=8), bv_bc)

        def emit_qk(xt_sb, q_sb, qc, f):
            pqk = ps_acc.tile([128, QC], FP32, name="pqk", tag="pv")
            for cc in range(CCH):
                nc.tensor.matmul(
                    out=pqk, lhsT=wqk_sb[:, cc, f], rhs=xt_sb[:, cc],
                    start=(cc == 0), stop=(cc == CCH - 1))
            p, isk = f // 2, f % 2
            dst = (ksb[p][:, qc * QC:(qc + 1) * QC] if isk else q_sb[p])
            nc.vector.tensor_scalar_add(dst, pqk, bqk_sb[:, f:f + 1])

        def emit_proj(y_qc, qc, oc):
            pp = ps_acc.tile([128, QC], FP32, name="pp", tag="pv")
            for p in range(NP):
                nc.tensor.matmul(out=pp, lhsT=wp_sb[:, p, oc], rhs=y_qc[p],
                                 start=(p == 0), stop=(p == NP - 1))
            po = op.tile([128, QC], FP32, name="po")
            nc.vector.tensor_scalar_add(po, pp, bp_sb[:, oc:oc + 1])
            # scalar-engine queue: keeps the bulk output writes off the
            # sync queue so the rz round trips there stay low-latency
            nc.scalar.dma_start(
                out=out_d[oc * 128:(oc + 1) * 128, qc * QC:(qc + 1) * QC],
                in_=po)

        def new_q():
            return [qp.tile([128, QC], BF16, name=f"qsb{p}", tag=f"qsb{p}")
                    for p in range(NP)]

        # ---------- prologue: chunk 0 qkv projection ----------
        xt_cur = load_xt(0, split=True)
        q_cur = new_q()
        for f in range(8):
            emit_qk(xt_cur, q_cur, 0, f)
        for j in range(QC // 128):
            emit_v(xt_cur, 0, j)

        y_prev = None
        for qc in range(nqc):
            # background PE units interleaved into this chunk's attention:
            # next chunk's qkv projection + previous chunk's output projection
            bg = []
            if qc + 1 < nqc:
                xt_nxt = load_xt(qc + 1)
                q_nxt = new_q()
                for j in range(QC // 128):
                    bg.append((emit_v, (xt_nxt, qc + 1, j)))
                for f in range(8):
                    bg.append((emit_qk, (xt_nxt, q_nxt, qc + 1, f)))
            else:
                xt_nxt, q_nxt = None, None
            if y_prev is not None:
                for oc in range(8):
                    bg.append((emit_proj, (y_prev, qc - 1, oc)))
            bg_total = len(bg)

            njb = 4 * (qc + 1)
            steps = NP * njb
            y_cur = [yp.tile([128, QC], FP32R, name=f"y{p}", tag=f"y{p}")
                     for p in range(NP)]
            step = 0
            for p in range(NP):
                yps = [ps_y.tile([HD + 1, QC], FP32, name=f"yps{e}", tag="yps")
                       for e in (0, 1)]
                for jb in range(njb):
                    r = jb - 4 * qc
                    cs = KB * r if r > 0 else 0
                    sps = ps_s.tile([128, 2, QC], FP32, name="sps")
                    for e in (0, 1):
                        nc.tensor.matmul(
                            out=sps[:, e, cs:],
                            lhsT=ksb[p][e * HD:(e + 1) * HD,
                                        jb * KB:(jb + 1) * KB],
                            rhs=q_cur[p][e * HD:(e + 1) * HD, cs:],
                            start=True, stop=True)
                    esb = ep.tile([128, 2, QC], BF16, name="esb")
                    nc.scalar.activation(out=esb[:, :, cs:], in_=sps[:, :, cs:],
                                         func=EXP, scale=0.125)
                    if r >= 0:
                        for e in (0, 1):
                            nc.vector.tensor_mul(
                                esb[:, e, cs:cs + KB],
                                esb[:, e, cs:cs + KB], mask)
                    for e in (0, 1):
                        nc.tensor.matmul(
                            out=yps[e][:, cs:], lhsT=vsb[jb][:, 2 * p + e, :],
                            rhs=esb[:, e, cs:],
                            start=(jb == 0), stop=(jb == njb - 1))
                    # keep the in-order PE stream dense: spread background
                    # units evenly across the attention steps
                    step += 1
                    while bg and len(bg) > bg_total * (steps - step) // steps:
                        fn, args = bg.pop(0)
                        fn(*args)
                # normalize: rz = 1/Z, broadcast over HD partitions via a
                # DRAM round trip (the sync DMA queue is quiet now that the
                # collective is gone, so the latency hides under attention)
                for e in (0, 1):
                    rz = sm.tile([1, QC], FP32, name="rz")
                    nc.vector.tensor_copy(rz, yps[e][HD:HD + 1, :])
                    nc.vector.reciprocal_approx_fast(rz, rz)
                    rzb = sm.tile([HD, QC], FP32, name="rzb")
                    slot = rz_dr[2 * (p % 2) + e]
                    # sync queue is kept free of bulk traffic so these small
                    # round trips stay low-latency
                    nc.sync.dma_start(out=slot, in_=rz[0:1, :])
                    nc.sync.dma_start(out=rzb, in_=slot.partition_broadcast(HD))
                    nc.vector.tensor_mul(
                        y_cur[p][e * HD:(e + 1) * HD, :], yps[e][0:HD, :], rzb)
            for fn, args in bg:
                fn(*args)
            y_prev = y_cur
            xt_cur, q_cur = xt_nxt, q_nxt

        # epilogue: last chunk's projection
        for oc in range(8):
            emit_proj(y_prev, nqc - 1, oc)

    nc.compile()
    return nc


def shard_inputs(x, W_attn, b_attn, W_proj, b_proj):
    in_maps = []
    u = np.arange(KB)[None, :]
    p_ = np.arange(128)[:, None]
    mask_np = (p_ <= u).astype(ml_dtypes.bfloat16)
    for c in range(NCORES):
        b, g = c // 2, c % 2
        xt = np.ascontiguousarray(x[b].T.astype(np.float32))
        # w_qk columns: feat chunk f = 2p+isK holds q (isK=0) or k (isK=1)
        # features of heads (8g+2p, 8g+2p+1)
        qk_idx = []
        for f in range(8):
            p, isk = f // 2, f % 2
            for e in (0, 1):
                h = 8 * g + 2 * p + e
                base = isk * C + h * HD
                qk_idx.append(np.arange(base, base + HD))
        qk_idx = np.concatenate(qk_idx)
        v_idx = np.concatenate(
            [np.arange(2 * C + (8 * g + l) * HD, 2 * C + (8 * g + l) * HD + HD)
             for l in range(8)])
        p_idx = np.concatenate(
            [np.arange((8 * g + l) * HD, (8 * g + l) * HD + HD)
             for l in range(8)])
        # pre-lay weights out as [128 partitions, flat free] matching the
        # SBUF resident tiles, so each loads as one wide-line DMA:
        #   wqk_sb [128, CCH, 8, 128]: [p, c, f, n] = W[c*128+p, f*128+n]
        wqk = W_attn[:, qk_idx].astype(np.float32)          # [C, C]
        wqk_h = wqk.reshape(CCH, 128, 8, 128).transpose(1, 0, 2, 3)
        wqk_h = np.ascontiguousarray(wqk_h.reshape(128, CCH * 8 * 128))
        wv = W_attn[:, v_idx].astype(np.float32)            # [C, 512]
        wv_h = wv.reshape(CCH, 128, 512).transpose(1, 0, 2)
        wv_h = np.ascontiguousarray(wv_h.reshape(128, CCH * 512))
        wp = W_proj[p_idx, :].astype(np.float32)            # [512, C]
        wp_h = wp.reshape(NP, 128, 8, 128).transpose(1, 0, 2, 3)
        wp_h = np.ascontiguousarray(wp_h.reshape(128, NP * 8 * 128))
        in_maps.append({
            "mask": mask_np,
            "xt": xt,
            "wqk": wqk_h,
            "bqk": np.ascontiguousarray(b_attn[qk_idx].astype(np.float32)),
            "wv": wv_h,
            "bv": np.ascontiguousarray(b_attn[v_idx].astype(np.float32)),
            "wp": wp_h,
            "bp": (b_proj.astype(np.float32) if g == 0
                   else np.zeros(C, np.float32)),
        })
    return in_maps


def assemble_output(results, T):
    out = np.empty((B, T, C), np.float32)
    for b in range(B):
        # each core holds a full [C, T] partial of out[b].T (row-parallel
        # projection); reduce the pair on host
        acc = results[2 * b]["out_t"] + results[2 * b + 1]["out_t"]
        out[b] = acc.T
    return out


_PROG = {}


def _get_program(T):
    if T not in _PROG:
        _PROG[T] = build_program(T)
    return _PROG[T]


def run_sharded(inputs, trace=False):
    """Returns (output [B,T,C], BassKernelResults)."""
    x = np.asarray(inputs["x"])
    T = x.shape[1]
    nc = _get_program(T)
    in_maps = shard_inputs(x, np.asarray(inputs["W_attn"]),
                           np.asarray(inputs["b_attn"]),
                           np.asarray(inputs["W_proj"]),
                           np.asarray(inputs["b_proj"]))
    res = run_bass_kernel_spmd(nc, in_maps, list(range(NCORES)), trace=trace)
    return assemble_output(res.results, T), res


def kernel(**inputs):
    out, _ = run_sharded(inputs)
    return out


# revision 22
# speedup vs baseline: 1.0471x; 1.0415x over previous
"""Causal self-attention kernel for 8 Trainium2 NeuronCores.

Problem: B=4, T=2048, C=1024, NH=16, HD=64 (fp32).

Sharding: core c = (batch b = c//2, head-group g = c%2 of 8 heads).
Per core, everything is computed in transposed layout so no on-device
transposes are needed:
  - host supplies xT = x[b].T [C, T], plus head-group-sliced/permuted
    weights (column-parallel W_attn, row-parallel W_proj)
  - qT/kT [feat, tok] via W stationary / xT moving; v [tok, feat] via
    xT stationary / W_v moving, with a fused ones-column per head so the
    attention row-sum (softmax denominator) falls out of the same matmul
  - scores are computed transposed [keys, queries] per 128-key block;
    exp on ACT writes bf16; causal masking = one [128,128] triangle
    multiply on the diagonal sub-block only; columns left of the diagonal
    sub-block (entirely masked) are clipped out of the score/exp/AV APs;
    blocks entirely above the diagonal are skipped
  - y^T accumulates in PSUM over key blocks; normalized by 1/Z where the
    1/Z row is partition-broadcast on GPSIMD (no DRAM round trip)
  - output projection is row-parallel -> partial out^T [C, T] DMA'd
    straight to DRAM; the pairwise reduction (+transpose) happens on host.

Matmuls run fp32r (fp22, full PE rate); q/k/esb/v are bf16.
"""

import numpy as np
import ml_dtypes
from contextlib import ExitStack

import concourse.bass as bass
import concourse.tile as tile
import concourse.mybir as mybir
from concourse import bacc
from concourse.bass_utils import run_bass_kernel_spmd

B, C, NH, HD = 4, 1024, 16, 64
NCORES = 8
NP = 4              # head pairs per core (8 heads)
QC = 512            # query-chunk (free dim of most matmuls)
KB = 128            # key block (partition dim of score blocks)
CCH = C // 128      # 8 contraction chunks
FP32 = mybir.dt.float32
FP32R = mybir.dt.float32r
BF16 = mybir.dt.bfloat16
EXP = mybir.ActivationFunctionType.Exp


def build_program(T=2048):
    nqc = T // QC
    nc = bacc.Bacc("TRN2", target_bir_lowering=False, debug=False,
                   num_devices=NCORES)

    xt_d = nc.dram_tensor("xt", [C, T], FP32, kind="ExternalInput").ap()
    # weights come host-pre-laid-out as [128, ...] so each loads as one
    # contiguous wide-line DMA (512B lines of the naive layout are ~4x
    # slower per the DMA efficiency curve)
    wqk_d = nc.dram_tensor("wqk", [128, CCH * 8 * 128], FP32, kind="ExternalInput").ap()
    bqk_d = nc.dram_tensor("bqk", [128, 8], FP32, kind="ExternalInput").ap()
    wv_d = nc.dram_tensor("wv", [128, CCH * 512], FP32, kind="ExternalInput").ap()
    bv_d = nc.dram_tensor("bv", [1, 512], FP32, kind="ExternalInput").ap()
    wp_d = nc.dram_tensor("wp", [128, NP * 8 * 128], FP32, kind="ExternalInput").ap()
    bp_d = nc.dram_tensor("bp", [128, 8], FP32, kind="ExternalInput").ap()
    mask_d = nc.dram_tensor("mask", [128, KB], BF16, kind="ExternalInput").ap()
    out_d = nc.dram_tensor("out_t", [C, T], FP32, kind="ExternalOutput").ap()
    rz_dr = [nc.dram_tensor(f"rzd{i}", [QC], FP32).ap() for i in range(4)]

    with tile.TileContext(nc) as tc, ExitStack() as ctx:
        resid = ctx.enter_context(tc.tile_pool(name="resid", bufs=1))
        xtp = ctx.enter_context(tc.tile_pool(name="xtp", bufs=2))
        qp = ctx.enter_context(tc.tile_pool(name="qp", bufs=2))
        yp = ctx.enter_context(tc.tile_pool(name="yp", bufs=2))
        ep = ctx.enter_context(tc.tile_pool(name="ep", bufs=3))
        sm = ctx.enter_context(tc.tile_pool(name="sm", bufs=3))
        op = ctx.enter_context(tc.tile_pool(name="op", bufs=2))
        ps_acc = ctx.enter_context(tc.tile_pool(name="ps_acc", bufs=2, space="PSUM"))
        ps_s = ctx.enter_context(tc.tile_pool(name="ps_s", bufs=2, space="PSUM"))
        ps_y = ctx.enter_context(tc.tile_pool(name="ps_y", bufs=2, space="PSUM"))

        # ---- residents: constants first (tiny, needed by early drains),
        # then wv (first PE work), then wqk, wp
        bqk_sb = resid.tile([128, 8], FP32, name="bqk_sb")
        nc.sync.dma_start(out=bqk_sb, in_=bqk_d)
        bp_sb = resid.tile([128, 8], FP32, name="bp_sb")
        nc.sync.dma_start(out=bp_sb, in_=bp_d)
        # bv_bc[:, l, :] = b_v (broadcast over partitions)
        bv_bc = resid.tile([128, 8, HD], FP32, name="bv_bc")
        nc.sync.dma_start(
            out=bv_bc,
            in_=bv_d[0].rearrange("(l d) -> l d", l=8).partition_broadcast(128))
        # causal triangle for the diagonal 128x128 sub-block:
        # mask[p, m] = 1.0 iff key-offset p <= query-offset m
        mask = resid.tile([128, KB], BF16, name="mask")
        nc.sync.dma_start(out=mask, in_=mask_d)

        wv_sb = resid.tile([128, CCH, 512], FP32R, name="wv_sb")
        nc.sync.dma_start(
            out=wv_sb,
            in_=wv_d.rearrange("p (c n) -> p c n", c=CCH).bitcast(FP32R))
        wqk_sb = resid.tile([128, CCH, 8, 128], FP32R, name="wqk_sb")
        nc.sync.dma_start(
            out=wqk_sb,
            in_=wqk_d.rearrange("p (c f n) -> p c f n", c=CCH, f=8).bitcast(FP32R))
        wp_sb = resid.tile([128, NP, 8, 128], FP32R, name="wp_sb")
        nc.sync.dma_start(
            out=wp_sb,
            in_=wp_d.rearrange("p (g f n) -> p g f n", g=NP, f=8).bitcast(FP32R))

        ksb = [resid.tile([128, T], BF16, name=f"ksb{p}") for p in range(NP)]
        vsb = [resid.tile([128, 8, HD + 1], BF16, name=f"vsb{tb}")
               for tb in range(T // 128)]
        for tb in range(T // 128):
            # ones column rides along into v so the attention row-sum Z
            # falls out of the same AV matmul
            nc.vector.memset(vsb[tb][:, :, HD:HD + 1], 1.0)

        # ---------- emission helpers ----------
        def load_xt(qc, split=False):
            xt_sb = xtp.tile([128, CCH, QC], FP32R, name="xt_sb")
            src = xt_d[:, qc * QC:(qc + 1) * QC].rearrange(
                "(c p) f -> p c f", p=128).bitcast(FP32R)
            if split:
                for cc in range(CCH):
                    nc.scalar.dma_start(out=xt_sb[:, cc], in_=src[:, cc])
            else:
                nc.scalar.dma_start(out=xt_sb, in_=src)
            return xt_sb

        def emit_v(xt_sb, qc, j):
            tb = qc * (QC // 128) + j
            pv = ps_acc.tile([128, 512], FP32, name="pv")
            for cc in range(CCH):
                nc.tensor.matmul(
                    out=pv, lhsT=xt_sb[:, cc, j * 128:(j + 1) * 128],
                    rhs=wv_sb[:, cc], start=(cc == 0), stop=(cc == CCH - 1))
            nc.vector.tensor_add(
                vsb[tb][:, :, 0:HD],
                pv.rearrange("p (l d) -> p l d", l=8), bv_bc)

        def emit_qk(xt_sb, q_sb, qc, f):
            pqk = ps_acc.tile([128, QC], FP32, name="pqk", tag="pv")
            for cc in range(CCH):
                nc.tensor.matmul(
                    out=pqk, lhsT=wqk_sb[:, cc, f], rhs=xt_sb[:, cc],
                    start=(cc == 0), stop=(cc == CCH - 1))
            p, isk = f // 2, f % 2
            dst = (ksb[p][:, qc * QC:(qc + 1) * QC] if isk else q_sb[p])
            nc.vector.tensor_scalar_add(dst, pqk, bqk_sb[:, f:f + 1])

        def emit_proj(y_qc, qc, oc):
            pp = ps_acc.tile([128, QC], FP32, name="pp", tag="pv")
            for p in range(NP):
                nc.tensor.matmul(out=pp, lhsT=wp_sb[:, p, oc], rhs=y_qc[p],
                                 start=(p == 0), stop=(p == NP - 1))
            po = op.tile([128, QC], FP32, name="po")
            nc.vector.tensor_scalar_add(po, pp, bp_sb[:, oc:oc + 1])
            # scalar-engine queue: keeps the bulk output writes off the
            # sync queue so the rz round trips there stay low-latency
            nc.scalar.dma_start(
                out=out_d[oc * 128:(oc + 1) * 128, qc * QC:(qc + 1) * QC],
                in_=po)

        def new_q():
            return [qp.tile([128, QC], BF16, name=f"qsb{p}", tag=f"qsb{p}")
                    for p in range(NP)]

        # ---------- prologue: chunk 0 qkv projection (v first: wv is the
        # first weight to land) ----------
        xt_cur = load_xt(0, split=True)
        q_cur = new_q()
        for j in range(QC // 128):
            emit_v(xt_cur, 0, j)
        for f in range(8):
            emit_qk(xt_cur, q_cur, 0, f)

        y_prev = None
        pending_norm = None
        for qc in range(nqc):
            # background PE units interleaved into this chunk's attention:
            # next chunk's qkv projection + previous chunk's output projection
            bg = []
            if qc + 1 < nqc:
                xt_nxt = load_xt(qc + 1)
                q_nxt = new_q()
                for j in range(QC // 128):
                    bg.append((emit_v, (xt_nxt, qc + 1, j)))
                for f in range(8):
                    bg.append((emit_qk, (xt_nxt, q_nxt, qc + 1, f)))
            else:
                xt_nxt, q_nxt = None, None
            if y_prev is not None:
                for oc in range(8):
                    bg.append((emit_proj, (y_prev, qc - 1, oc)))
            bg_total = len(bg)

            njb = 4 * (qc + 1)
            steps = NP * njb
            y_cur = [yp.tile([128, QC], FP32R, name=f"y{p}", tag=f"y{p}")
                     for p in range(NP)]
            step = 0
            for p in range(NP):
                yps = [ps_y.tile([HD + 1, QC], FP32, name=f"yps{e}", tag="yps")
                       for e in (0, 1)]
                # (AV deferral past the previous group's normalize passed
                # CoreSim but raced on HW — disabled)
                defer = 0
                if pending_norm is not None:
                    pending_norm()
                    pending_norm = None
                deferred_av = []
                for jb in range(njb):
                    r = jb - 4 * qc
                    cs = KB * r if r > 0 else 0
                    sps = ps_s.tile([128, 2, QC], FP32, name="sps")
                    for e in (0, 1):
                        nc.tensor.matmul(
                            out=sps[:, e, cs:],
                            lhsT=ksb[p][e * HD:(e + 1) * HD,
                                        jb * KB:(jb + 1) * KB],
                            rhs=q_cur[p][e * HD:(e + 1) * HD, cs:],
                            start=True, stop=True)
                    esb = ep.tile([128, 2, QC], BF16, name="esb")
                    nc.scalar.activation(out=esb[:, :, cs:], in_=sps[:, :, cs:],
                                         func=EXP, scale=0.125)
                    if r >= 0:
                        for e in (0, 1):
                            nc.vector.tensor_mul(
                                esb[:, e, cs:cs + KB],
                                esb[:, e, cs:cs + KB], mask)

                    def emit_av(jb=jb, cs=cs, esb=esb, yps=yps, p=p,
                                njb=njb):
                        for e in (0, 1):
                            nc.tensor.matmul(
                                out=yps[e][:, cs:],
                                lhsT=vsb[jb][:, 2 * p + e, :],
                                rhs=esb[:, e, cs:],
                                start=(jb == 0), stop=(jb == njb - 1))

                    if jb < defer:
                        deferred_av.append(emit_av)
                        if jb == defer - 1:
                            pending_norm()
                            pending_norm = None
                            for fn in deferred_av:
                                fn()
                            deferred_av = []
                    else:
                        emit_av()
                    # keep the in-order PE stream dense: spread background
                    # units evenly across the attention steps
                    step += 1
                    while bg and len(bg) > bg_total * (steps - step) // steps:
                        fn, args = bg.pop(0)
                        fn(*args)

                # normalize: rz = 1/Z, broadcast over HD partitions via a
                # DRAM round trip; deferred into the next p-group so its
                # latency overlaps that group's score/exp front
                def make_norm(p=p, yps=yps, y_t=y_cur[p]):
                    def norm():
                        for e in (0, 1):
                            rz = sm.tile([1, QC], FP32, name="rz")
                            nc.vector.tensor_copy(rz, yps[e][HD:HD + 1, :])
                            nc.vector.reciprocal_approx_fast(rz, rz)
                            rzb = sm.tile([HD, QC], FP32, name="rzb")
                            slot = rz_dr[2 * (p % 2) + e]
                            nc.sync.dma_start(out=slot, in_=rz[0:1, :])
                            nc.sync.dma_start(
                                out=rzb, in_=slot.partition_broadcast(HD))
                            nc.vector.tensor_mul(
                                y_t[e * HD:(e + 1) * HD, :],
                                yps[e][0:HD, :], rzb)
                    return norm
                pending_norm = make_norm()
            for fn, args in bg:
                fn(*args)
            y_prev = y_cur
            xt_cur, q_cur = xt_nxt, q_nxt

        # epilogue: last chunk's projection
        if pending_norm is not None:
            pending_norm()
            pending_norm = None
        for oc in range(8):
            emit_proj(y_prev, nqc - 1, oc)

    nc.compile()
    return nc


def shard_inputs(x, W_attn, b_attn, W_proj, b_proj):
    in_maps = []
    u = np.arange(KB)[None, :]
    p_ = np.arange(128)[:, None]
    mask_np = (p_ <= u).astype(ml_dtypes.bfloat16)
    for c in range(NCORES):
        b, g = c // 2, c % 2
        xt = np.ascontiguousarray(x[b].T.astype(np.float32))
        # w_qk columns: feat chunk f = 2p+isK holds q (isK=0) or k (isK=1)
        # features of heads (8g+2p, 8g+2p+1)
        qk_idx = []
        for f in range(8):
            p, isk = f // 2, f % 2
            for e in (0, 1):
                h = 8 * g + 2 * p + e
                base = isk * C + h * HD
                qk_idx.append(np.arange(base, base + HD))
        qk_idx = np.concatenate(qk_idx)
        v_idx = np.concatenate(
            [np.arange(2 * C + (8 * g + l) * HD, 2 * C + (8 * g + l) * HD + HD)
             for l in range(8)])
        p_idx = np.concatenate(
            [np.arange((8 * g + l) * HD, (8 * g + l) * HD + HD)
             for l in range(8)])
        # pre-lay weights out as [128 partitions, flat free] matching the
        # SBUF resident tiles, so each loads as one wide-line DMA:
        #   wqk_sb [128, CCH, 8, 128]: [p, c, f, n] = W[c*128+p, f*128+n]
        wqk = W_attn[:, qk_idx].astype(np.float32)          # [C, C]
        wqk_h = wqk.reshape(CCH, 128, 8, 128).transpose(1, 0, 2, 3)
        wqk_h = np.ascontiguousarray(wqk_h.reshape(128, CCH * 8 * 128))
        wv = W_attn[:, v_idx].astype(np.float32)            # [C, 512]
        wv_h = wv.reshape(CCH, 128, 512).transpose(1, 0, 2)
        wv_h = np.ascontiguousarray(wv_h.reshape(128, CCH * 512))
        wp = W_proj[p_idx, :].astype(np.float32)            # [512, C]
        wp_h = wp.reshape(NP, 128, 8, 128).transpose(1, 0, 2, 3)
        wp_h = np.ascontiguousarray(wp_h.reshape(128, NP * 8 * 128))
        bqk = b_attn[qk_idx].astype(np.float32)
        bqk_h = np.ascontiguousarray(bqk.reshape(8, 128).T)      # [128, 8]
        bp = (b_proj.astype(np.float32) if g == 0
              else np.zeros(C, np.float32))
        bp_h = np.ascontiguousarray(bp.reshape(8, 128).T)        # [128, 8]
        bv_h = np.ascontiguousarray(
            b_attn[v_idx].astype(np.float32).reshape(1, 512))
        in_maps.append({
            "mask": mask_np,
            "xt": xt,
            "wqk": wqk_h,
            "bqk": bqk_h,
            "wv": wv_h,
            "bv": bv_h,
            "wp": wp_h,
            "bp": bp_h,
        })
    return in_maps


def assemble_output(results, T):
    out = np.empty((B, T, C), np.float32)
    for b in range(B):
        # each core holds a full [C, T] partial of out[b].T (row-parallel
        # projection); reduce the pair on host
        acc = results[2 * b]["out_t"] + results[2 * b + 1]["out_t"]
        out[b] = acc.T
    return out


_PROG = {}


def _get_program(T):
    if T not in _PROG:
        _PROG[T] = build_program(T)
    return _PROG[T]


def run_sharded(inputs, trace=False):
    """Returns (output [B,T,C], BassKernelResults)."""
    x = np.asarray(inputs["x"])
    T = x.shape[1]
    nc = _get_program(T)
    in_maps = shard_inputs(x, np.asarray(inputs["W_attn"]),
                           np.asarray(inputs["b_attn"]),
                           np.asarray(inputs["W_proj"]),
                           np.asarray(inputs["b_proj"]))
    res = run_bass_kernel_spmd(nc, in_maps, list(range(NCORES)), trace=trace)
    return assemble_output(res.results, T), res


def kernel(**inputs):
    out, _ = run_sharded(inputs)
    return out


# revision 32
# speedup vs baseline: 1.1006x; 1.0511x over previous
"""Causal self-attention kernel for 8 Trainium2 NeuronCores.

Problem: B=4, T=2048, C=1024, NH=16, HD=64 (fp32).

Sharding: core c = (batch b = c//2, head-group g = c%2 of 8 heads).
Per core, everything is computed in transposed layout so no on-device
transposes are needed:
  - host supplies xT = x[b].T [C, T], plus head-group-sliced/permuted
    weights (column-parallel W_attn, row-parallel W_proj)
  - qT/kT [feat, tok] via W stationary / xT moving; v [tok, feat] via
    xT stationary / W_v moving, with a fused ones-column per head so the
    attention row-sum (softmax denominator) falls out of the same matmul
  - scores are computed transposed [keys, queries] per 128-key block;
    exp on ACT writes bf16; causal masking = one [128,128] triangle
    multiply on the diagonal sub-block only; columns left of the diagonal
    sub-block (entirely masked) are clipped out of the score/exp/AV APs;
    blocks entirely above the diagonal are skipped
  - y^T accumulates in PSUM over key blocks; normalized by 1/Z where the
    1/Z row is partition-broadcast on GPSIMD (no DRAM round trip)
  - output projection is row-parallel -> partial out^T [C, T] DMA'd
    straight to DRAM; the pairwise reduction (+transpose) happens on host.

Matmuls run fp32r (fp22, full PE rate); q/k/esb/v are bf16.
"""

import numpy as np
import ml_dtypes
from contextlib import ExitStack

import concourse.bass as bass
import concourse.tile as tile
import concourse.mybir as mybir
from concourse import bacc
from concourse.bass_utils import run_bass_kernel_spmd

B, C, NH, HD = 4, 1024, 16, 64
NCORES = 8
NP = 4              # head pairs per core (8 heads)
QC = 512            # query-chunk (free dim of most matmuls)
KB = 128            # key block (partition dim of score blocks)
CCH = C // 128      # 8 contraction chunks
FP32 = mybir.dt.float32
FP32R = mybir.dt.float32r
BF16 = mybir.dt.bfloat16
EXP = mybir.ActivationFunctionType.Exp


def build_program(T=2048):
    nqc = T // QC
    nc = bacc.Bacc("TRN2", target_bir_lowering=False, debug=False,
                   num_devices=NCORES)

    # xt comes chunk-major, host-pre-transposed to the SBUF tile layout so
    # each chunk loads as one contiguous 16KB-line DMA
    xt_d = nc.dram_tensor("xt", [T // QC, 128, CCH * QC], FP32,
                          kind="ExternalInput").ap()
    # weights come host-pre-laid-out as [128, ...] so each loads as one
    # contiguous wide-line DMA (512B lines of the naive layout are ~4x
    # slower per the DMA efficiency curve)
    wqk_d = nc.dram_tensor("wqk", [128, CCH * 8 * 128], FP32, kind="ExternalInput").ap()
    bqk_d = nc.dram_tensor("bqk", [128, 8], FP32, kind="ExternalInput").ap()
    wv_d = nc.dram_tensor("wv", [128, CCH * 512], FP32, kind="ExternalInput").ap()
    bv_d = nc.dram_tensor("bv", [1, 512], FP32, kind="ExternalInput").ap()
    wp_d = nc.dram_tensor("wp", [128, NP * 8 * 128], FP32, kind="ExternalInput").ap()
    bp_d = nc.dram_tensor("bp", [128, 8], FP32, kind="ExternalInput").ap()
    mask_d = nc.dram_tensor("mask", [128, KB], BF16, kind="ExternalInput").ap()
    out_d = nc.dram_tensor("out_t", [C, T], FP32, kind="ExternalOutput").ap()
    rz_dr = [nc.dram_tensor(f"rzd{i}", [QC], FP32).ap() for i in range(4)]

    with tile.TileContext(nc) as tc, ExitStack() as ctx:
        resid = ctx.enter_context(tc.tile_pool(name="resid", bufs=1))
        xtp = ctx.enter_context(tc.tile_pool(name="xtp", bufs=2))
        qp = ctx.enter_context(tc.tile_pool(name="qp", bufs=2))
        yp = ctx.enter_context(tc.tile_pool(name="yp", bufs=2))
        ep = ctx.enter_context(tc.tile_pool(name="ep", bufs=3))
        sm = ctx.enter_context(tc.tile_pool(name="sm", bufs=3))
        op = ctx.enter_context(tc.tile_pool(name="op", bufs=2))
        ps_acc = ctx.enter_context(tc.tile_pool(name="ps_acc", bufs=2, space="PSUM"))
        ps_s = ctx.enter_context(tc.tile_pool(name="ps_s", bufs=2, space="PSUM"))
        ps_y = ctx.enter_context(tc.tile_pool(name="ps_y", bufs=2, space="PSUM"))

        # ---- residents: constants first (tiny, needed by early drains),
        # then wv (first PE work), then wqk, wp
        bqk_sb = resid.tile([128, 8], FP32, name="bqk_sb")
        nc.sync.dma_start(out=bqk_sb, in_=bqk_d)
        bp_sb = resid.tile([128, 8], FP32, name="bp_sb")
        nc.sync.dma_start(out=bp_sb, in_=bp_d)
        # bv_bc[:, l, :] = b_v (broadcast over partitions)
        bv_bc = resid.tile([128, 8, HD], FP32, name="bv_bc")
        nc.sync.dma_start(
            out=bv_bc,
            in_=bv_d[0].rearrange("(l d) -> l d", l=8).partition_broadcast(128))
        # causal triangle for the diagonal 128x128 sub-block:
        # mask[p, m] = 1.0 iff key-offset p <= query-offset m
        mask = resid.tile([128, KB], BF16, name="mask")
        nc.sync.dma_start(out=mask, in_=mask_d)

        wv_sb = resid.tile([128, CCH, 512], FP32R, name="wv_sb")
        nc.sync.dma_start(
            out=wv_sb,
            in_=wv_d.rearrange("p (c n) -> p c n", c=CCH).bitcast(FP32R))
        # wqk split into f-halves across both queues so emit_qk's first
        # f-groups can start while the second half still streams
        wqk_sb = resid.tile([128, CCH, 8, 128], FP32R, name="wqk_sb")
        wqk_src = wqk_d.rearrange("p (c f n) -> p c f n", c=CCH, f=8).bitcast(FP32R)
        nc.sync.dma_start(out=wqk_sb[:, :, 0:4], in_=wqk_src[:, :, 0:4])
        nc.scalar.dma_start(out=wqk_sb[:, :, 4:8], in_=wqk_src[:, :, 4:8])
        wp_sb = resid.tile([128, NP, 8, 128], FP32R, name="wp_sb")
        nc.sync.dma_start(
            out=wp_sb,
            in_=wp_d.rearrange("p (g f n) -> p g f n", g=NP, f=8).bitcast(FP32R))

        ksb = [resid.tile([128, T], BF16, name=f"ksb{p}") for p in range(NP)]
        vsb = [resid.tile([128, 8, HD + 1], BF16, name=f"vsb{tb}")
               for tb in range(T // 128)]
        for tb in range(T // 128):
            # ones column rides along into v so the attention row-sum Z
            # falls out of the same AV matmul
            nc.vector.memset(vsb[tb][:, :, HD:HD + 1], 1.0)

        # ---------- emission helpers ----------
        def load_xt(qc, split=False):
            xt_sb = xtp.tile([128, CCH, QC], FP32R, name="xt_sb")
            src = xt_d[qc].rearrange("p (c f) -> p c f", c=CCH).bitcast(FP32R)
            if split:
                for h in (0, 1):
                    nc.scalar.dma_start(out=xt_sb[:, 4 * h:4 * h + 4],
                                        in_=src[:, 4 * h:4 * h + 4])
            else:
                nc.scalar.dma_start(out=xt_sb, in_=src)
            return xt_sb

        def emit_v(xt_sb, qc, j):
            tb = qc * (QC // 128) + j
            pv = ps_acc.tile([128, 512], FP32, name="pv")
            for cc in range(CCH):
                nc.tensor.matmul(
                    out=pv, lhsT=xt_sb[:, cc, j * 128:(j + 1) * 128],
                    rhs=wv_sb[:, cc], start=(cc == 0), stop=(cc == CCH - 1))
            nc.vector.tensor_add(
                vsb[tb][:, :, 0:HD],
                pv.rearrange("p (l d) -> p l d", l=8), bv_bc)

        def emit_qk(xt_sb, q_sb, qc, f):
            pqk = ps_acc.tile([128, QC], FP32, name="pqk", tag="pv")
            for cc in range(CCH):
                nc.tensor.matmul(
                    out=pqk, lhsT=wqk_sb[:, cc, f], rhs=xt_sb[:, cc],
                    start=(cc == 0), stop=(cc == CCH - 1))
            p, isk = f // 2, f % 2
            dst = (ksb[p][:, qc * QC:(qc + 1) * QC] if isk else q_sb[p])
            nc.vector.tensor_scalar_add(dst, pqk, bqk_sb[:, f:f + 1])

        def emit_proj(y_qc, qc, oc):
            pp = ps_acc.tile([128, QC], FP32, name="pp", tag="pv")
            for p in range(NP):
                nc.tensor.matmul(out=pp, lhsT=wp_sb[:, p, oc], rhs=y_qc[p],
                                 start=(p == 0), stop=(p == NP - 1))
            po = op.tile([128, QC], FP32, name="po")
            nc.vector.tensor_scalar_add(po, pp, bp_sb[:, oc:oc + 1])
            # scalar-engine queue: keeps the bulk output writes off the
            # sync queue so the rz round trips there stay low-latency
            nc.scalar.dma_start(
                out=out_d[oc * 128:(oc + 1) * 128, qc * QC:(qc + 1) * QC],
                in_=po)

        def new_q():
            return [qp.tile([128, QC], BF16, name=f"qsb{p}", tag=f"qsb{p}")
                    for p in range(NP)]

        # ---------- prologue: chunk 0 qkv projection (v first: wv is the
        # first weight to land) ----------
        xt_cur = load_xt(0, split=True)
        xt_pre = load_xt(1) if nqc > 1 else None
        q_cur = new_q()
        for j in range(QC // 128):
            emit_v(xt_cur, 0, j)
        for f in range(8):
            emit_qk(xt_cur, q_cur, 0, f)

        y_prev = None
        pending_norm = None
        for qc in range(nqc):
            # background PE units interleaved into this chunk's attention:
            # next chunk's qkv projection + previous chunk's output
            # projection. xt is prefetched two chunks deep so the qkv units
            # are always data-ready when the pacing pops them.
            bg = []
            if qc + 1 < nqc:
                xt_nxt = xt_pre
                xt_pre = load_xt(qc + 2) if qc + 2 < nqc else None
                q_nxt = new_q()
                for j in range(QC // 128):
                    bg.append((emit_v, (xt_nxt, qc + 1, j)))
                for f in range(8):
                    bg.append((emit_qk, (xt_nxt, q_nxt, qc + 1, f)))
            else:
                xt_nxt, q_nxt = None, None
            if y_prev is not None:
                for oc in range(8):
                    bg.append((emit_proj, (y_prev, qc - 1, oc)))
            bg_total = len(bg)

            njb = 4 * (qc + 1)
            steps = NP * njb
            y_cur = [yp.tile([128, QC], FP32R, name=f"y{p}", tag=f"y{p}")
                     for p in range(NP)]
            step = 0
            for p in range(NP):
                # yps tiles are allocated LAZILY at the first AV emission:
                # tile() records buffer-reuse deps against instructions
                # emitted so far, so allocating before the (deferred)
                # previous-group normalize races its yps reads on HW
                yps_box = []

                def get_yps(yps_box=yps_box):
                    if not yps_box:
                        yps_box.extend(
                            ps_y.tile([HD + 1, QC], FP32, name=f"yps{e}",
                                      tag="yps")
                            for e in (0, 1))
                    return yps_box
                # (deferring AVs past the previous group's normalize fails
                # deterministically on HW — flush the normalize up front and
                # instead hide its latency by stuffing ready background units
                # between this group's first score/exp and first AV)
                defer = 0
                stuff = 0
                if pending_norm is not None:
                    pending_norm()
                    pending_norm = None
                    stuff = 2
                deferred_av = []
                for jb in range(njb):
                    r = jb - 4 * qc
                    cs = KB * r if r > 0 else 0
                    sps = ps_s.tile([128, 2, QC], FP32, name="sps")
                    for e in (0, 1):
                        nc.tensor.matmul(
                            out=sps[:, e, cs:],
                            lhsT=ksb[p][e * HD:(e + 1) * HD,
                                        jb * KB:(jb + 1) * KB],
                            rhs=q_cur[p][e * HD:(e + 1) * HD, cs:],
                            start=True, stop=True)
                    esb = ep.tile([128, 2, QC], BF16, name="esb")
                    nc.scalar.activation(out=esb[:, :, cs:], in_=sps[:, :, cs:],
                                         func=EXP, scale=0.125)
                    if r >= 0:
                        for e in (0, 1):
                            nc.vector.tensor_mul(
                                esb[:, e, cs:cs + KB],
                                esb[:, e, cs:cs + KB], mask)

                    def emit_av(jb=jb, cs=cs, esb=esb, p=p, njb=njb,
                                get_yps=get_yps):
                        yps = get_yps()
                        for e in (0, 1):
                            nc.tensor.matmul(
                                out=yps[e][:, cs:],
                                lhsT=vsb[jb][:, 2 * p + e, :],
                                rhs=esb[:, e, cs:],
                                start=(jb == 0), stop=(jb == njb - 1))

                    if jb == 0 and stuff:
                        # background units are independent of yps, so they
                        # fill the PE while the previous group's normalize
                        # chain frees the yps buffers this group's AVs need
                        for _ in range(stuff):
                            if bg:
                                fn, args = bg.pop(0)
                                fn(*args)
                    if jb < defer:
                        deferred_av.append(emit_av)
                    else:
                        emit_av()
                    # keep the in-order PE stream dense: spread background
                    # units evenly across the attention steps
                    step += 1
                    while bg and len(bg) > bg_total * (steps - step) // steps:
                        fn, args = bg.pop(0)
                        fn(*args)

                # normalize: rz = 1/Z, broadcast over HD partitions via a
                # DRAM round trip; deferred into the next p-group so its
                # latency overlaps that group's score/exp front
                def make_norm(p=p, yps=get_yps(), y_t=y_cur[p]):
                    def norm():
                        for e in (0, 1):
                            rz = sm.tile([1, QC], FP32, name="rz")
                            nc.vector.tensor_copy(rz, yps[e][HD:HD + 1, :])
                            nc.vector.reciprocal_approx_fast(rz, rz)
                            rzb = sm.tile([HD, QC], FP32, name="rzb")
                            slot = rz_dr[2 * (p % 2) + e]
                            nc.sync.dma_start(out=slot, in_=rz[0:1, :])
                            nc.sync.dma_start(
                                out=rzb, in_=slot.partition_broadcast(HD))
                            nc.vector.tensor_mul(
                                y_t[e * HD:(e + 1) * HD, :],
                                yps[e][0:HD, :], rzb)
                    return norm
                pending_norm = make_norm()
            for fn, args in bg:
                fn(*args)
            y_prev = y_cur
            xt_cur, q_cur = xt_nxt, q_nxt

        # epilogue: last chunk's projection
        if pending_norm is not None:
            pending_norm()
            pending_norm = None
        for oc in range(8):
            emit_proj(y_prev, nqc - 1, oc)

    nc.compile()
    return nc


def shard_inputs(x, W_attn, b_attn, W_proj, b_proj):
    in_maps = []
    u = np.arange(KB)[None, :]
    p_ = np.arange(128)[:, None]
    mask_np = (p_ <= u).astype(ml_dtypes.bfloat16)
    T = x.shape[1]
    nqc = T // QC
    for c in range(NCORES):
        b, g = c // 2, c % 2
        # chunk-major xt matching the SBUF tile: [qc][p, c*QC+f] =
        # x[b, qc*QC+f, c*128+p] — loads as contiguous 16KB-line DMAs
        xb = x[b].astype(np.float32)                       # [T, C]
        xt = xb.reshape(nqc, QC, CCH, 128).transpose(0, 3, 2, 1)
        xt = np.ascontiguousarray(xt.reshape(nqc, 128, CCH * QC))
        # w_qk columns: feat chunk f = 2p+isK holds q (isK=0) or k (isK=1)
        # features of heads (8g+2p, 8g+2p+1)
        qk_idx = []
        for f in range(8):
            p, isk = f // 2, f % 2
            for e in (0, 1):
                h = 8 * g + 2 * p + e
                base = isk * C + h * HD
                qk_idx.append(np.arange(base, base + HD))
        qk_idx = np.concatenate(qk_idx)
        v_idx = np.concatenate(
            [np.arange(2 * C + (8 * g + l) * HD, 2 * C + (8 * g + l) * HD + HD)
             for l in range(8)])
        p_idx = np.concatenate(
            [np.arange((8 * g + l) * HD, (8 * g + l) * HD + HD)
             for l in range(8)])
        # pre-lay weights out as [128 partitions, flat free] matching the
        # SBUF resident tiles, so each loads as one wide-line DMA:
        #   wqk_sb [128, CCH, 8, 128]: [p, c, f, n] = W[c*128+p, f*128+n]
        wqk = W_attn[:, qk_idx].astype(np.float32)          # [C, C]
        wqk_h = wqk.reshape(CCH, 128, 8, 128).transpose(1, 0, 2, 3)
        wqk_h = np.ascontiguousarray(wqk_h.reshape(128, CCH * 8 * 128))
        wv = W_attn[:, v_idx].astype(np.float32)            # [C, 512]
        wv_h = wv.reshape(CCH, 128, 512).transpose(1, 0, 2)
        wv_h = np.ascontiguousarray(wv_h.reshape(128, CCH * 512))
        wp = W_proj[p_idx, :].astype(np.float32)            # [512, C]
        wp_h = wp.reshape(NP, 128, 8, 128).transpose(1, 0, 2, 3)
        wp_h = np.ascontiguousarray(wp_h.reshape(128, NP * 8 * 128))
        bqk = b_attn[qk_idx].astype(np.float32)
        bqk_h = np.ascontiguousarray(bqk.reshape(8, 128).T)      # [128, 8]
        bp = (b_proj.astype(np.float32) if g == 0
              else np.zeros(C, np.float32))
        bp_h = np.ascontiguousarray(bp.reshape(8, 128).T)        # [128, 8]
        bv_h = np.ascontiguousarray(
            b_attn[v_idx].astype(np.float32).reshape(1, 512))
        in_maps.append({
            "mask": mask_np,
            "xt": xt,
            "wqk": wqk_h,
            "bqk": bqk_h,
            "wv": wv_h,
            "bv": bv_h,
            "wp": wp_h,
            "bp": bp_h,
        })
    return in_maps


def assemble_output(results, T):
    out = np.empty((B, T, C), np.float32)
    for b in range(B):
        # each core holds a full [C, T] partial of out[b].T (row-parallel
        # projection); reduce the pair on host
        acc = results[2 * b]["out_t"] + results[2 * b + 1]["out_t"]
        out[b] = acc.T
    return out


_PROG = {}


def _get_program(T):
    if T not in _PROG:
        _PROG[T] = build_program(T)
    return _PROG[T]


def run_sharded(inputs, trace=False):
    """Returns (output [B,T,C], BassKernelResults)."""
    x = np.asarray(inputs["x"])
    T = x.shape[1]
    nc = _get_program(T)
    in_maps = shard_inputs(x, np.asarray(inputs["W_attn"]),
                           np.asarray(inputs["b_attn"]),
                           np.asarray(inputs["W_proj"]),
                           np.asarray(inputs["b_proj"]))
    res = run_bass_kernel_spmd(nc, in_maps, list(range(NCORES)), trace=trace)
    return assemble_output(res.results, T), res


def kernel(**inputs):
    out, _ = run_sharded(inputs)
    return out
